# revision 15
# baseline (speedup 1.0000x reference)
"""Binarized BasicBlock (BNN) forward on 8 Trainium2 NeuronCores.

Reference computation (per reference.py):
    xb  = sign(x);  wb = sign(w)
    y1  = conv3x3(xb, wb1, pad=1)
    a1  = hardtanh(bn1(y1))          # only sign(a1) feeds conv2
    y2  = conv3x3(sign(a1), wb2, pad=1)
    out = hardtanh(bn2(y2) + x)

Strategy:
  - Data parallel: batch N=64 -> 8 images per core; weights/BN replicated.
  - Conv as 9 shifted matmuls over a zero-padded 58x58 image held in SBUF,
    contraction over input channels: 256 channels = 2 planes of 128
    partitions contracted in ONE matmul via fp8 DoubleRow perf mode.
  - Input is binarized AND laid out into the padded row-interleaved SBUF
    image format ON THE HOST; the kernel just DMAs it. This removes the
    ScalarE sign pass + pad memsets from the critical path and lets conv1
    start as soon as the first DMA lands (PE/HAM stays warm end to end).
  - Binarized operands stored as fp8e4 (+-1, 0 exact); PSUM accumulates
    fp32; sums of +-1 with <=2304 terms are exact integers in fp32.
  - BN folded into the activation op: sign(bn1(y)) = Sign(y*s1 + t1) with
    s1 = g1/sqrt(v1+eps), t1 = b1 - m1*s1 (host-folded, passed as inputs).
  - Final stage: Identity(y2*s2+t2) on ScalarE, add-residual (fp32 -> bf16)
    and clip on VectorE, bf16 chunk stores (host upconverts to fp32).
  - DMA queues: SP carries xb/residual loads (+ last image's stores so the
    tail rides the low-latency HWDGE); Pool(SWDGE) carries weights/BN and
    steady-state stores, so prefetch loads never queue behind stores.
"""

import sys

try:
    import concourse  # noqa: F401
except ImportError:  # pragma: no cover
    sys.path.insert(0, "/opt/trn_rl_repo")

import numpy as np
import ml_dtypes

import concourse.bacc as bacc
import concourse.tile as tile
import concourse.mybir as mybir
from concourse.bass_utils import run_bass_kernel_spmd

dt = mybir.dt
AF = mybir.ActivationFunctionType
ALU = mybir.AluOpType
PM = mybir.MatmulPerfMode

N_CORES = 8
NPER = 8          # images per core
C = 256
H = W = 56
HW = H * W        # 3136
ROWW = 64         # allocated width per (row, k-plane) block (16B aligned)
RPITCH = 2 * ROWW  # 128 = row pitch (both k-planes interleaved per row)
PROWS = 58        # padded rows
PLSZ = PROWS * RPITCH  # 7424 = padded image tile length
RPC = 8           # output rows per matmul chunk
CHU = RPC * W     # 448 = useful matmul free dim
NCH = H // RPC    # 7 chunks per image
BN_EPS = 1e-5

_CACHE = {}


def _zero_pads(nc, t):
    """Zero the padding cells of a [128, PLSZ] row-interleaved image tile.

    Layout: element (row r, k-plane k, col c) at r*RPITCH + k*ROWW + c;
    c=1..56 hold image cols 0..55, c=0 and c=57..63 are zero pads, rows
    0 and 57 are zero pad rows."""
    v = t[:]
    nc.gpsimd.memset(v[:, 0:RPITCH], 0.0)                      # top pad row
    nc.gpsimd.memset(v[:, 57 * RPITCH:PLSZ], 0.0)              # bottom pad row
    # per-block right pads c=57..63 plus the following block's c=0
    cols = v[:, 57:57 + 57 * RPITCH].rearrange("p (r k c) -> p r k c", k=2, c=ROWW)
    nc.gpsimd.memset(cols[:, :, :, 0:8], 0.0)


def _rview(t):
    # [128, PROWS, 2, ROWW]
    return t[:].rearrange("p (r k c) -> p r k c", k=2, c=ROWW)


def _build():
    nc = bacc.Bacc("TRN2", target_bir_lowering=False, debug=False)

    # two host-prepared copies of each padded image: copy 0 has image cols at
    # c=1..56, copy 1 at c=2..57 — so every conv1 tap reads an even-byte AP
    # start (odd starts cost +7ns per matmul on the PE SBUF read path).
    xb_d = nc.dram_tensor("xb", [NPER, 2, 128, PLSZ], dt.float8e4, kind="ExternalInput").ap()
    res_d = nc.dram_tensor("res", [NPER, 2, 128, HW], dt.float32, kind="ExternalInput").ap()
    w1_d = nc.dram_tensor("w1b", [128, 2, 9, C], dt.float8e4, kind="ExternalInput").ap()
    w2_d = nc.dram_tensor("w2b", [128, 2, 9, C], dt.float8e4, kind="ExternalInput").ap()
    s1_d = nc.dram_tensor("s1", [128, 2], dt.float32, kind="ExternalInput").ap()
    t1_d = nc.dram_tensor("t1", [128, 2], dt.float32, kind="ExternalInput").ap()
    s2_d = nc.dram_tensor("s2", [128, 2], dt.float32, kind="ExternalInput").ap()
    t2_d = nc.dram_tensor("t2", [128, 2], dt.float32, kind="ExternalInput").ap()
    out_d = nc.dram_tensor("out", [NPER, 2, 128, HW], dt.bfloat16, kind="ExternalOutput").ap()

    with tile.TileContext(nc) as tc:
        with (
            tc.tile_pool(name="wp", bufs=1) as wp,
            tc.tile_pool(name="xin", bufs=2) as xinp,
            tc.tile_pool(name="xb", bufs=3) as xbp,
            tc.tile_pool(name="ab", bufs=2) as abp,
            tc.tile_pool(name="ost", bufs=6) as ostp,
            tc.tile_pool(name="tmp", bufs=4) as tmpp,
            tc.tile_pool(name="ps", bufs=7, space="PSUM") as psp,
            nc.sbuf_tensor([128, 2 * CHU], dt.float8e4) as warm_in,
            nc.psum_tensor([128, CHU], dt.float32) as warm_ps,
        ):
            # Queue layout at the head: SP(hwdge) carries the image-0 xb load
            # (the true head dependency), ScalarE(hwdge) carries w1, Pool
            # (SWDGE) does the warm-up memset first and then w2 + BN vectors.
            w_sb = [
                wp.tile([128, 2, 9, C], dt.float8e4, tag=tag, name=f"w_{tag}")
                for tag in ("w1", "w2")
            ]
            bn_sb = [
                wp.tile([128, 2], dt.float32, tag=tag, name=f"bn_{tag}")
                for tag in ("s1", "t1", "s2", "t2")
            ]
            s1_sb, t1_sb, s2_sb, t2_sb = bn_sb

            xb_tiles = []
            xb = xbp.tile([128, 2, PLSZ], dt.float8e4, tag="xb")
            xb_tiles.append(xb)
            # image-0 load as interleaved row-range pieces (both copies per
            # range) with w1 tap groups woven in, all on the SP HWDGE ring,
            # ordered by when the PE first needs each piece.
            w1_groups = [(0, 3), (3, 6), (6, 9)]
            head_order = [
                ("xb", 0, 0, 14), ("xb", 1, 0, 14), ("w1", 0, 0, 3),
                ("xb", 0, 14, 26), ("xb", 1, 14, 26), ("w1", 1, 0, 3),
                ("xb", 0, 26, 42), ("xb", 1, 26, 42), ("w1", 2, 0, 3),
                ("xb", 0, 42, 58), ("xb", 1, 42, 58),
            ]
            for item in head_order:
                if item[0] == "xb":
                    _, a, r0, r1 = item
                    nc.sync.dma_start(
                        xb[:, a, r0 * RPITCH:r1 * RPITCH],
                        xb_d[0, a, :, r0 * RPITCH:r1 * RPITCH])
                else:
                    k0, k1 = w1_groups[item[1]]
                    nc.sync.dma_start(w_sb[0][:, :, k0:k1], w1_d[:, :, k0:k1])

            # PE warm-up: junk matmuls on (uninitialized) scratch ramp HAM to
            # 8/8 (~4.7us of sustained PE activity) while the image-0 xb DMA
            # and w1 land; sized so real matmuls follow warm with no PE gap.
            wv = warm_in[:].rearrange("p (k c) -> p k c", k=2)
            for _ in range(15):
                nc.tensor.matmul(
                    warm_ps[:], wv[:, :, 0:128], wv[:],
                    start=True, stop=True, perf_mode=PM.DoubleRow,
                )

            nc.gpsimd.dma_start(w_sb[1][:], w2_d)
            for t, bd in zip(bn_sb, (s1_d, t1_d, s2_d, t2_d)):
                nc.gpsimd.dma_start(t[:], bd)

            for n in range(NPER):
                # ---- prefetch binarized input (already padded on host) ----
                if n == 0:
                    xb = xb_tiles[0]
                else:
                    xb = xbp.tile([128, 2, PLSZ], dt.float8e4, tag="xb")
                    nc.sync.dma_start(xb[:], xb_d[n].rearrange("a p f -> p a f"))
                xbv = xb[:].rearrange("p a (r k c) -> p a r k c", k=2, c=ROWW)
                xin = xinp.tile([128, 2, HW], dt.float32, tag="xin")
                nc.sync.dma_start(xin[:], res_d[n].rearrange("q p f -> p q f"))

                # ---- conv1 -> sign(bn1(.)) into padded intermediate ----
                ab = abp.tile([128, PLSZ], dt.float8e4, tag="ab")
                if n < 2:
                    _zero_pads(nc, ab)  # pads stay zero across reuses
                abv = _rview(ab)
                for co in range(2):
                    for s in range(NCH):
                        ps = psp.tile([128, CHU], dt.float32, tag="ps")
                        for kk in range(9):
                            r0 = RPC * s + kk // 3
                            kw = kk % 3
                            a, c0 = (1, 2) if kw == 1 else (0, kw)
                            rhs = xbv[:, a, r0:r0 + RPC, :, c0:c0 + W].rearrange(
                                "p r k c -> p k r c")
                            nc.tensor.matmul(
                                ps[:],
                                w_sb[0][:, :, kk, co * 128:(co + 1) * 128],
                                rhs,
                                start=(kk == 0),
                                stop=(kk == 8),
                                perf_mode=PM.DoubleRow,
                            )
                        psv = ps[:].rearrange("p (r c) -> p r c", c=W)
                        nc.scalar.activation(
                            abv[:, 1 + RPC * s:1 + RPC * s + RPC, co, 1:57], psv, AF.Sign,
                            bias=t1_sb[:, co:co + 1], scale=s1_sb[:, co:co + 1],
                        )

                # ---- conv2 -> bn2 + residual + clip -> bf16 chunk stores ----
                for co in range(2):
                    xinv = xin[:, co].rearrange("p (h w) -> p h w", w=W)
                    # the very last chunk is split into two 4-row PSUM chunks
                    # so the final epilogue overlaps the preceding matmuls
                    chunks = [(RPC * s, RPC) for s in range(NCH)]
                    if n == NPER - 1 and co == 1:
                        chunks[-1:] = [(48, 4), (52, 4)]
                    for ci, (cr0, cnr) in enumerate(chunks):
                        ps = psp.tile([128, CHU], dt.float32, tag="ps")
                        for kk in range(9):
                            r0 = cr0 + kk // 3
                            rhs = abv[:, r0:r0 + cnr, :, kk % 3:kk % 3 + W].rearrange(
                                "p r k c -> p k r c")
                            nc.tensor.matmul(
                                ps[:, 0:cnr * W],
                                w_sb[1][:, :, kk, co * 128:(co + 1) * 128],
                                rhs,
                                start=(kk == 0),
                                stop=(kk == 8),
                                perf_mode=PM.DoubleRow,
                            )
                        psv = ps[:, 0:cnr * W].rearrange("p (r c) -> p r c", c=W)
                        tm = tmpp.tile([128, CHU], dt.float32, tag="tmp")
                        tmv = tm[:, 0:cnr * W].rearrange("p (r c) -> p r c", c=W)
                        nc.scalar.activation(
                            tmv, psv, AF.Identity,
                            bias=t2_sb[:, co:co + 1], scale=s2_sb[:, co:co + 1],
                        )
                        oc = ostp.tile([128, CHU], dt.bfloat16, tag="ost")
                        ocv = oc[:, 0:cnr * W].rearrange("p (r c) -> p r c", c=W)
                        nc.vector.tensor_tensor(
                            ocv, tmv, xinv[:, cr0:cr0 + cnr, :], ALU.add
                        )
                        nc.vector.tensor_scalar(
                            oc[:, 0:cnr * W], oc[:, 0:cnr * W],
                            1.0, -1.0, ALU.min, ALU.max)
                        st_eng = nc.sync if n == NPER - 1 else nc.gpsimd
                        st_eng.dma_start(
                            out_d[n, co, :, cr0 * W:(cr0 + cnr) * W],
                            oc[:, 0:cnr * W],
                        )

    nc.compile()
    return nc


def _get_nc():
    if "nc" not in _CACHE:
        _CACHE["nc"] = _build()
    return _CACHE["nc"]


def _prep_weights(w):
    # [co, cin, kh, kw] -> [cin 128, cin_chunk 2, tap 9, co 256], binarized fp8e4
    a = np.sign(w.astype(np.float32))
    a = a.transpose(1, 2, 3, 0).reshape(2, 128, 9, C).transpose(1, 0, 2, 3)
    return np.ascontiguousarray(a.astype(ml_dtypes.float8_e4m3))


def _fold_bn(g, b, m, v):
    s = (g.astype(np.float32) / np.sqrt(v.astype(np.float32) + BN_EPS)).astype(np.float32)
    t = (b.astype(np.float32) - m.astype(np.float32) * s).astype(np.float32)
    return (
        np.ascontiguousarray(s.reshape(2, 128).T),
        np.ascontiguousarray(t.reshape(2, 128).T),
    )


def _prep_x(x):
    """[64,256,56,56] fp32 -> padded binarized [64,2,128,58,2,64] fp8e4.

    Copy 0 holds image cols at c=1..56, copy 1 at c=2..57 (even-byte AP
    starts for every conv1 tap)."""
    n = x.shape[0]
    xb = np.zeros((n, 2, 128, PROWS, 2, ROWW), dtype=ml_dtypes.float8_e4m3)
    s = np.sign(x).astype(ml_dtypes.float8_e4m3)
    # [n, 2, 128, 56, 56] -> [n, 128, 56, 2, 56]
    s = s.reshape(n, 2, 128, H, W).transpose(0, 2, 3, 1, 4)
    xb[:, 0, :, 1:57, :, 1:57] = s
    xb[:, 1, :, 1:57, :, 2:58] = s
    return xb.reshape(n, 2, 128, PLSZ)


def _make_in_maps(inputs):
    x = inputs["x"]
    w1b = _prep_weights(inputs["w1"])
    w2b = _prep_weights(inputs["w2"])
    s1, t1 = _fold_bn(inputs["g1"], inputs["b1"], inputs["m1"], inputs["v1"])
    s2, t2 = _fold_bn(inputs["g2"], inputs["b2"], inputs["m2"], inputs["v2"])
    x = np.ascontiguousarray(x.astype(np.float32, copy=False))
    xb = _prep_x(x)
    res = x.reshape(N_CORES * NPER, 2, 128, HW)
    in_maps = []
    for c in range(N_CORES):
        in_maps.append({
            "xb": xb[c * NPER:(c + 1) * NPER],
            "res": res[c * NPER:(c + 1) * NPER],
            "w1b": w1b, "w2b": w2b,
            "s1": s1, "t1": t1, "s2": s2, "t2": t2,
        })
    return in_maps


def kernel(x, w1, g1, b1, m1, v1, w2, g2, b2, m2, v2):
    nc = _get_nc()
    in_maps = _make_in_maps({
        "x": x, "w1": w1, "g1": g1, "b1": b1, "m1": m1, "v1": v1,
        "w2": w2, "g2": g2, "b2": b2, "m2": m2, "v2": v2,
    })
    res = run_bass_kernel_spmd(nc, in_maps, list(range(N_CORES)))
    out = np.concatenate([res.results[c]["out"] for c in range(N_CORES)], axis=0)
    return out.astype(np.float32).reshape(N_CORES * NPER, C, H, W)


# revision 16
# speedup vs baseline: 1.0014x; 1.0014x over previous
"""Binarized BasicBlock (BNN) forward on 8 Trainium2 NeuronCores.

Reference computation (per reference.py):
    xb  = sign(x);  wb = sign(w)
    y1  = conv3x3(xb, wb1, pad=1)
    a1  = hardtanh(bn1(y1))          # only sign(a1) feeds conv2
    y2  = conv3x3(sign(a1), wb2, pad=1)
    out = hardtanh(bn2(y2) + x)

Strategy:
  - Data parallel: batch N=64 -> 8 images per core; weights/BN replicated.
  - Conv as 9 shifted matmuls over a zero-padded 58x58 image held in SBUF,
    contraction over input channels: 256 channels = 2 planes of 128
    partitions contracted in ONE matmul via fp8 DoubleRow perf mode.
  - Input is binarized AND laid out into the padded row-interleaved SBUF
    image format ON THE HOST; the kernel just DMAs it. This removes the
    ScalarE sign pass + pad memsets from the critical path and lets conv1
    start as soon as the first DMA lands (PE/HAM stays warm end to end).
  - Binarized operands stored as fp8e4 (+-1, 0 exact); PSUM accumulates
    fp32; sums of +-1 with <=2304 terms are exact integers in fp32.
  - BN folded into the activation op: sign(bn1(y)) = Sign(y*s1 + t1) with
    s1 = g1/sqrt(v1+eps), t1 = b1 - m1*s1 (host-folded, passed as inputs).
  - Final stage: Identity(y2*s2+t2) on ScalarE, add-residual (fp32 -> bf16)
    and clip on VectorE, bf16 chunk stores (host upconverts to fp32).
  - DMA queues: SP carries xb/residual loads (+ last image's stores so the
    tail rides the low-latency HWDGE); Pool(SWDGE) carries weights/BN and
    steady-state stores, so prefetch loads never queue behind stores.
"""

import sys

try:
    import concourse  # noqa: F401
except ImportError:  # pragma: no cover
    sys.path.insert(0, "/opt/trn_rl_repo")

import numpy as np
import ml_dtypes

import concourse.bacc as bacc
import concourse.tile as tile
import concourse.mybir as mybir
from concourse.bass_utils import run_bass_kernel_spmd

dt = mybir.dt
AF = mybir.ActivationFunctionType
ALU = mybir.AluOpType
PM = mybir.MatmulPerfMode

N_CORES = 8
NPER = 8          # images per core
C = 256
H = W = 56
HW = H * W        # 3136
ROWW = 64         # allocated width per (row, k-plane) block (16B aligned)
RPITCH = 2 * ROWW  # 128 = row pitch (both k-planes interleaved per row)
PROWS = 58        # padded rows
PLSZ = PROWS * RPITCH  # 7424 = padded image tile length
RPC = 8           # output rows per matmul chunk
CHU = RPC * W     # 448 = useful matmul free dim
NCH = H // RPC    # 7 chunks per image
BN_EPS = 1e-5

_CACHE = {}


def _zero_pads(nc, t):
    """Zero the padding cells of a [128, PLSZ] row-interleaved image tile.

    Layout: element (row r, k-plane k, col c) at r*RPITCH + k*ROWW + c;
    c=1..56 hold image cols 0..55, c=0 and c=57..63 are zero pads, rows
    0 and 57 are zero pad rows."""
    v = t[:]
    nc.gpsimd.memset(v[:, 0:RPITCH], 0.0)                      # top pad row
    nc.gpsimd.memset(v[:, 57 * RPITCH:PLSZ], 0.0)              # bottom pad row
    # per-block right pads c=57..63 plus the following block's c=0
    cols = v[:, 57:57 + 57 * RPITCH].rearrange("p (r k c) -> p r k c", k=2, c=ROWW)
    nc.gpsimd.memset(cols[:, :, :, 0:8], 0.0)


def _rview(t):
    # [128, PROWS, 2, ROWW]
    return t[:].rearrange("p (r k c) -> p r k c", k=2, c=ROWW)


def _build():
    nc = bacc.Bacc("TRN2", target_bir_lowering=False, debug=False)

    # two host-prepared copies of each padded image: copy 0 has image cols at
    # c=1..56, copy 1 at c=2..57 — so every conv1 tap reads an even-byte AP
    # start (odd starts cost +7ns per matmul on the PE SBUF read path).
    xb_d = nc.dram_tensor("xb", [NPER, 2, 128, PLSZ], dt.float8e4, kind="ExternalInput").ap()
    res_d = nc.dram_tensor("res", [NPER, 2, 128, HW], dt.float32, kind="ExternalInput").ap()
    w1_d = nc.dram_tensor("w1b", [128, 2, 9, C], dt.float8e4, kind="ExternalInput").ap()
    w2_d = nc.dram_tensor("w2b", [128, 2, 9, C], dt.float8e4, kind="ExternalInput").ap()
    s1_d = nc.dram_tensor("s1", [128, 2], dt.float32, kind="ExternalInput").ap()
    t1_d = nc.dram_tensor("t1", [128, 2], dt.float32, kind="ExternalInput").ap()
    s2_d = nc.dram_tensor("s2", [128, 2], dt.float32, kind="ExternalInput").ap()
    t2_d = nc.dram_tensor("t2", [128, 2], dt.float32, kind="ExternalInput").ap()
    out_d = nc.dram_tensor("out", [NPER, 2, 128, HW], dt.bfloat16, kind="ExternalOutput").ap()

    with tile.TileContext(nc) as tc:
        with (
            tc.tile_pool(name="wp", bufs=1) as wp,
            tc.tile_pool(name="xin", bufs=2) as xinp,
            tc.tile_pool(name="xb", bufs=3) as xbp,
            tc.tile_pool(name="ab", bufs=2) as abp,
            tc.tile_pool(name="ost", bufs=6) as ostp,
            tc.tile_pool(name="tmp", bufs=4) as tmpp,
            tc.tile_pool(name="ps", bufs=7, space="PSUM") as psp,
            nc.sbuf_tensor([128, 2 * CHU], dt.float8e4) as warm_in,
            nc.psum_tensor([128, CHU], dt.float32) as warm_ps,
        ):
            # Queue layout at the head: SP(hwdge) carries the image-0 xb load
            # (the true head dependency), ScalarE(hwdge) carries w1, Pool
            # (SWDGE) does the warm-up memset first and then w2 + BN vectors.
            w_sb = [
                wp.tile([128, 2, 9, C], dt.float8e4, tag=tag, name=f"w_{tag}")
                for tag in ("w1", "w2")
            ]
            bn_sb = [
                wp.tile([128, 2], dt.float32, tag=tag, name=f"bn_{tag}")
                for tag in ("s1", "t1", "s2", "t2")
            ]
            s1_sb, t1_sb, s2_sb, t2_sb = bn_sb

            xb_tiles = []
            xb = xbp.tile([128, 2, PLSZ], dt.float8e4, tag="xb")
            xb_tiles.append(xb)
            # image-0 load as interleaved row-range pieces (both copies per
            # range) on the SP ring; w1 as ONE dma on the ScalarE ring (a
            # reader of the tile waits for ALL its writer pieces, so one
            # piece on a parallel ring completes earliest, ~10.5us).
            for r0, r1 in ((0, 14), (14, 26), (26, 42), (42, 58)):
                for a in range(2):
                    nc.sync.dma_start(
                        xb[:, a, r0 * RPITCH:r1 * RPITCH],
                        xb_d[0, a, :, r0 * RPITCH:r1 * RPITCH])
            nc.scalar.dma_start(w_sb[0][:], w1_d)

            # PE warm-up: junk matmuls on (uninitialized) scratch ramp HAM to
            # 8/8 (~4.7us of sustained PE activity) while the image-0 xb DMA
            # and w1 land; sized so real matmuls follow with no PE gap.
            wv = warm_in[:].rearrange("p (k c) -> p k c", k=2)
            for _ in range(11):
                nc.tensor.matmul(
                    warm_ps[:], wv[:, :, 0:128], wv[:],
                    start=True, stop=True, perf_mode=PM.DoubleRow,
                )

            nc.gpsimd.dma_start(w_sb[1][:], w2_d)
            for t, bd in zip(bn_sb, (s1_d, t1_d, s2_d, t2_d)):
                nc.gpsimd.dma_start(t[:], bd)

            for n in range(NPER):
                # ---- prefetch binarized input (already padded on host) ----
                if n == 0:
                    xb = xb_tiles[0]
                else:
                    xb = xbp.tile([128, 2, PLSZ], dt.float8e4, tag="xb")
                    nc.sync.dma_start(xb[:], xb_d[n].rearrange("a p f -> p a f"))
                xbv = xb[:].rearrange("p a (r k c) -> p a r k c", k=2, c=ROWW)
                xin = xinp.tile([128, 2, HW], dt.float32, tag="xin")
                nc.sync.dma_start(xin[:], res_d[n].rearrange("q p f -> p q f"))

                # ---- conv1 -> sign(bn1(.)) into padded intermediate ----
                ab = abp.tile([128, PLSZ], dt.float8e4, tag="ab")
                if n < 2:
                    _zero_pads(nc, ab)  # pads stay zero across reuses
                abv = _rview(ab)
                for co in range(2):
                    for s in range(NCH):
                        ps = psp.tile([128, CHU], dt.float32, tag="ps")
                        for kk in range(9):
                            r0 = RPC * s + kk // 3
                            kw = kk % 3
                            a, c0 = (1, 2) if kw == 1 else (0, kw)
                            rhs = xbv[:, a, r0:r0 + RPC, :, c0:c0 + W].rearrange(
                                "p r k c -> p k r c")
                            nc.tensor.matmul(
                                ps[:],
                                w_sb[0][:, :, kk, co * 128:(co + 1) * 128],
                                rhs,
                                start=(kk == 0),
                                stop=(kk == 8),
                                perf_mode=PM.DoubleRow,
                            )
                        psv = ps[:].rearrange("p (r c) -> p r c", c=W)
                        nc.scalar.activation(
                            abv[:, 1 + RPC * s:1 + RPC * s + RPC, co, 1:57], psv, AF.Sign,
                            bias=t1_sb[:, co:co + 1], scale=s1_sb[:, co:co + 1],
                        )

                # ---- conv2 -> bn2 + residual + clip -> bf16 chunk stores ----
                for co in range(2):
                    xinv = xin[:, co].rearrange("p (h w) -> p h w", w=W)
                    # the very last chunk is split into two 4-row PSUM chunks
                    # so the final epilogue overlaps the preceding matmuls
                    chunks = [(RPC * s, RPC) for s in range(NCH)]
                    if n == NPER - 1 and co == 1:
                        chunks[-1:] = [(48, 4), (52, 4)]
                    for ci, (cr0, cnr) in enumerate(chunks):
                        ps = psp.tile([128, CHU], dt.float32, tag="ps")
                        for kk in range(9):
                            r0 = cr0 + kk // 3
                            rhs = abv[:, r0:r0 + cnr, :, kk % 3:kk % 3 + W].rearrange(
                                "p r k c -> p k r c")
                            nc.tensor.matmul(
                                ps[:, 0:cnr * W],
                                w_sb[1][:, :, kk, co * 128:(co + 1) * 128],
                                rhs,
                                start=(kk == 0),
                                stop=(kk == 8),
                                perf_mode=PM.DoubleRow,
                            )
                        psv = ps[:, 0:cnr * W].rearrange("p (r c) -> p r c", c=W)
                        tm = tmpp.tile([128, CHU], dt.float32, tag="tmp")
                        tmv = tm[:, 0:cnr * W].rearrange("p (r c) -> p r c", c=W)
                        nc.scalar.activation(
                            tmv, psv, AF.Identity,
                            bias=t2_sb[:, co:co + 1], scale=s2_sb[:, co:co + 1],
                        )
                        oc = ostp.tile([128, CHU], dt.bfloat16, tag="ost")
                        ocv = oc[:, 0:cnr * W].rearrange("p (r c) -> p r c", c=W)
                        nc.vector.tensor_tensor(
                            ocv, tmv, xinv[:, cr0:cr0 + cnr, :], ALU.add
                        )
                        nc.vector.tensor_scalar(
                            oc[:, 0:cnr * W], oc[:, 0:cnr * W],
                            1.0, -1.0, ALU.min, ALU.max)
                        st_eng = nc.sync if n == NPER - 1 else nc.gpsimd
                        st_eng.dma_start(
                            out_d[n, co, :, cr0 * W:(cr0 + cnr) * W],
                            oc[:, 0:cnr * W],
                        )

    nc.compile()
    return nc


def _get_nc():
    if "nc" not in _CACHE:
        _CACHE["nc"] = _build()
    return _CACHE["nc"]


def _prep_weights(w):
    # [co, cin, kh, kw] -> [cin 128, cin_chunk 2, tap 9, co 256], binarized fp8e4
    a = np.sign(w.astype(np.float32))
    a = a.transpose(1, 2, 3, 0).reshape(2, 128, 9, C).transpose(1, 0, 2, 3)
    return np.ascontiguousarray(a.astype(ml_dtypes.float8_e4m3))


def _fold_bn(g, b, m, v):
    s = (g.astype(np.float32) / np.sqrt(v.astype(np.float32) + BN_EPS)).astype(np.float32)
    t = (b.astype(np.float32) - m.astype(np.float32) * s).astype(np.float32)
    return (
        np.ascontiguousarray(s.reshape(2, 128).T),
        np.ascontiguousarray(t.reshape(2, 128).T),
    )


def _prep_x(x):
    """[64,256,56,56] fp32 -> padded binarized [64,2,128,58,2,64] fp8e4.

    Copy 0 holds image cols at c=1..56, copy 1 at c=2..57 (even-byte AP
    starts for every conv1 tap)."""
    n = x.shape[0]
    xb = np.zeros((n, 2, 128, PROWS, 2, ROWW), dtype=ml_dtypes.float8_e4m3)
    s = np.sign(x).astype(ml_dtypes.float8_e4m3)
    # [n, 2, 128, 56, 56] -> [n, 128, 56, 2, 56]
    s = s.reshape(n, 2, 128, H, W).transpose(0, 2, 3, 1, 4)
    xb[:, 0, :, 1:57, :, 1:57] = s
    xb[:, 1, :, 1:57, :, 2:58] = s
    return xb.reshape(n, 2, 128, PLSZ)


def _make_in_maps(inputs):
    x = inputs["x"]
    w1b = _prep_weights(inputs["w1"])
    w2b = _prep_weights(inputs["w2"])
    s1, t1 = _fold_bn(inputs["g1"], inputs["b1"], inputs["m1"], inputs["v1"])
    s2, t2 = _fold_bn(inputs["g2"], inputs["b2"], inputs["m2"], inputs["v2"])
    x = np.ascontiguousarray(x.astype(np.float32, copy=False))
    xb = _prep_x(x)
    res = x.reshape(N_CORES * NPER, 2, 128, HW)
    in_maps = []
    for c in range(N_CORES):
        in_maps.append({
            "xb": xb[c * NPER:(c + 1) * NPER],
            "res": res[c * NPER:(c + 1) * NPER],
            "w1b": w1b, "w2b": w2b,
            "s1": s1, "t1": t1, "s2": s2, "t2": t2,
        })
    return in_maps


def kernel(x, w1, g1, b1, m1, v1, w2, g2, b2, m2, v2):
    nc = _get_nc()
    in_maps = _make_in_maps({
        "x": x, "w1": w1, "g1": g1, "b1": b1, "m1": m1, "v1": v1,
        "w2": w2, "g2": g2, "b2": b2, "m2": m2, "v2": v2,
    })
    res = run_bass_kernel_spmd(nc, in_maps, list(range(N_CORES)))
    out = np.concatenate([res.results[c]["out"] for c in range(N_CORES)], axis=0)
    return out.astype(np.float32).reshape(N_CORES * NPER, C, H, W)


# revision 21
# speedup vs baseline: 1.0020x; 1.0006x over previous
"""Binarized BasicBlock (BNN) forward on 8 Trainium2 NeuronCores.

Reference computation (per reference.py):
    xb  = sign(x);  wb = sign(w)
    y1  = conv3x3(xb, wb1, pad=1)
    a1  = hardtanh(bn1(y1))          # only sign(a1) feeds conv2
    y2  = conv3x3(sign(a1), wb2, pad=1)
    out = hardtanh(bn2(y2) + x)

Strategy:
  - Data parallel: batch N=64 -> 8 images per core; weights/BN replicated.
  - Conv as 9 shifted matmuls over a zero-padded 58x58 image held in SBUF,
    contraction over input channels: 256 channels = 2 planes of 128
    partitions contracted in ONE matmul via fp8 DoubleRow perf mode.
  - Input is binarized AND laid out into the padded row-interleaved SBUF
    image format ON THE HOST; the kernel just DMAs it. This removes the
    ScalarE sign pass + pad memsets from the critical path and lets conv1
    start as soon as the first DMA lands (PE/HAM stays warm end to end).
  - Binarized operands stored as fp8e4 (+-1, 0 exact); PSUM accumulates
    fp32; sums of +-1 with <=2304 terms are exact integers in fp32.
  - BN folded into the activation op: sign(bn1(y)) = Sign(y*s1 + t1) with
    s1 = g1/sqrt(v1+eps), t1 = b1 - m1*s1 (host-folded, passed as inputs).
  - Final stage: Identity(y2*s2+t2) on ScalarE, add-residual (fp32 -> bf16)
    and clip on VectorE, bf16 chunk stores (host upconverts to fp32).
  - DMA queues: SP carries xb/residual loads (+ last image's stores so the
    tail rides the low-latency HWDGE); Pool(SWDGE) carries weights/BN and
    steady-state stores, so prefetch loads never queue behind stores.
"""

import sys

try:
    import concourse  # noqa: F401
except ImportError:  # pragma: no cover
    sys.path.insert(0, "/opt/trn_rl_repo")

import numpy as np
import ml_dtypes

import concourse.bacc as bacc
import concourse.tile as tile
import concourse.mybir as mybir
from concourse.bass_utils import run_bass_kernel_spmd

dt = mybir.dt
AF = mybir.ActivationFunctionType
ALU = mybir.AluOpType
PM = mybir.MatmulPerfMode

N_CORES = 8
NPER = 8          # images per core
C = 256
H = W = 56
HW = H * W        # 3136
ROWW = 64         # allocated width per (row, k-plane) block (16B aligned)
RPITCH = 2 * ROWW  # 128 = row pitch (both k-planes interleaved per row)
PROWS = 58        # padded rows
PLSZ = PROWS * RPITCH  # 7424 = padded image tile length
RPC = 8           # output rows per matmul chunk
CHU = RPC * W     # 448 = useful matmul free dim
NCH = H // RPC    # 7 chunks per image
BN_EPS = 1e-5

_CACHE = {}


def _zero_pads(nc, t):
    """Zero the padding cells of a [128, PLSZ] row-interleaved image tile.

    Layout: element (row r, k-plane k, col c) at r*RPITCH + k*ROWW + c;
    c=1..56 hold image cols 0..55, c=0 and c=57..63 are zero pads, rows
    0 and 57 are zero pad rows."""
    v = t[:]
    nc.gpsimd.memset(v[:, 0:RPITCH], 0.0)                      # top pad row
    nc.gpsimd.memset(v[:, 57 * RPITCH:PLSZ], 0.0)              # bottom pad row
    # per-block right pads c=57..63 plus the following block's c=0
    cols = v[:, 57:57 + 57 * RPITCH].rearrange("p (r k c) -> p r k c", k=2, c=ROWW)
    nc.gpsimd.memset(cols[:, :, :, 0:8], 0.0)


def _rview(t):
    # [128, PROWS, 2, ROWW]
    return t[:].rearrange("p (r k c) -> p r k c", k=2, c=ROWW)


def _build():
    nc = bacc.Bacc("TRN2", target_bir_lowering=False, debug=False)

    # two host-prepared copies of each padded image: copy 0 has image cols at
    # c=1..56, copy 1 at c=2..57 — so every conv1 tap reads an even-byte AP
    # start (odd starts cost +7ns per matmul on the PE SBUF read path).
    xb_d = nc.dram_tensor("xb", [NPER, 2, 128, PLSZ], dt.float8e4, kind="ExternalInput").ap()
    res_d = nc.dram_tensor("res", [NPER, 2, 128, HW], dt.float32, kind="ExternalInput").ap()
    w1_d = nc.dram_tensor("w1b", [128, 2, 9, C], dt.float8e4, kind="ExternalInput").ap()
    w2_d = nc.dram_tensor("w2b", [128, 2, 9, C], dt.float8e4, kind="ExternalInput").ap()
    s1_d = nc.dram_tensor("s1", [128, 2], dt.float32, kind="ExternalInput").ap()
    t1_d = nc.dram_tensor("t1", [128, 2], dt.float32, kind="ExternalInput").ap()
    s2_d = nc.dram_tensor("s2", [128, 2], dt.float32, kind="ExternalInput").ap()
    t2_d = nc.dram_tensor("t2", [128, 2], dt.float32, kind="ExternalInput").ap()
    out_d = nc.dram_tensor("out", [NPER, 2, 128, HW], dt.bfloat16, kind="ExternalOutput").ap()

    with tile.TileContext(nc) as tc:
        with (
            tc.tile_pool(name="wp", bufs=1) as wp,
            tc.tile_pool(name="xin", bufs=2) as xinp,
            tc.tile_pool(name="xb", bufs=3) as xbp,
            tc.tile_pool(name="ab", bufs=2) as abp,
            tc.tile_pool(name="ost", bufs=6) as ostp,
            tc.tile_pool(name="tmp", bufs=4) as tmpp,
            tc.tile_pool(name="ps", bufs=7, space="PSUM") as psp,
            nc.sbuf_tensor([128, 2 * CHU], dt.float8e4) as warm_in,
            nc.psum_tensor([128, CHU], dt.float32) as warm_ps,
        ):
            # Queue layout at the head: SP(hwdge) carries the image-0 xb load
            # (the true head dependency), ScalarE(hwdge) carries w1, Pool
            # (SWDGE) does the warm-up memset first and then w2 + BN vectors.
            w_sb = [
                wp.tile([128, 2, 9, C], dt.float8e4, tag=tag, name=f"w_{tag}")
                for tag in ("w1", "w2")
            ]
            bn_sb = [
                wp.tile([128, 2], dt.float32, tag=tag, name=f"bn_{tag}")
                for tag in ("s1", "t1", "s2", "t2")
            ]
            s1_sb, t1_sb, s2_sb, t2_sb = bn_sb

            # Image-0 rows land in three SEPARATE tiles (a tile's reader
            # waits for ALL its writer DMAs, so small early tiles unblock the
            # first chunks at ~10us). Row ranges overlap so every chunk's
            # taps stay within one tile: A rows 0-17 (chunks 0-1), B rows
            # 16-33 (chunks 2-3), C rows 32-57 (chunks 4-6).
            xb0_parts = []
            for tag, pr0, pr1 in (("xb0a", 0, 18), ("xb0b", 16, 34), ("xb0c", 32, 58)):
                t = wp.tile([128, 2, (pr1 - pr0) * RPITCH], dt.float8e4,
                            tag=tag, name=f"t_{tag}")
                nc.sync.dma_start(
                    t[:], xb_d[0, :, :, pr0 * RPITCH:pr1 * RPITCH].rearrange(
                        "a p f -> p a f"))
                xb0_parts.append((pr0, t))

            # w1 first on the Pool SWDGE ring (no ACT_TABLE_LOAD ahead of it
            # there) — lands ~10us
            nc.gpsimd.dma_start(w_sb[0][:], w1_d)

            # PE warm-up: junk matmuls on (uninitialized) scratch ramp HAM to
            # 8/8 (~4.7us of sustained PE activity) while the image-0 xb DMA
            # and w1 land; sized so real matmuls follow with no PE gap.
            wv = warm_in[:].rearrange("p (k c) -> p k c", k=2)
            for _ in range(11):
                nc.tensor.matmul(
                    warm_ps[:], wv[:, :, 0:128], wv[:],
                    start=True, stop=True, perf_mode=PM.DoubleRow,
                )

            nc.gpsimd.dma_start(w_sb[1][:], w2_d)
            for t, bd in zip(bn_sb, (s1_d, t1_d, s2_d, t2_d)):
                nc.gpsimd.dma_start(t[:], bd)

            for n in range(NPER):
                # ---- prefetch binarized input (already padded on host) ----
                if n == 0:
                    xbv = None
                else:
                    xb = xbp.tile([128, 2, PLSZ], dt.float8e4, tag="xb")
                    nc.sync.dma_start(xb[:], xb_d[n].rearrange("a p f -> p a f"))
                    xbv = xb[:].rearrange("p a (r k c) -> p a r k c", k=2, c=ROWW)
                xin = xinp.tile([128, 2, HW], dt.float32, tag="xin")
                nc.sync.dma_start(xin[:], res_d[n].rearrange("q p f -> p q f"))

                # ---- conv1 -> sign(bn1(.)) into padded intermediate ----
                ab = abp.tile([128, PLSZ], dt.float8e4, tag="ab")
                if n < 2:
                    _zero_pads(nc, ab)  # pads stay zero across reuses
                abv = _rview(ab)
                for co in range(2):
                    for s in range(NCH):
                        ps = psp.tile([128, CHU], dt.float32, tag="ps")
                        if n == 0:
                            pr0, pt = xb0_parts[0 if s < 2 else (1 if s < 4 else 2)]
                            xv = pt[:].rearrange(
                                "p a (r k c) -> p a r k c", k=2, c=ROWW)
                        else:
                            pr0, xv = 0, xbv
                        for kk in range(9):
                            r0 = RPC * s + kk // 3 - pr0
                            kw = kk % 3
                            a, c0 = (1, 2) if kw == 1 else (0, kw)
                            rhs = xv[:, a, r0:r0 + RPC, :, c0:c0 + W].rearrange(
                                "p r k c -> p k r c")
                            nc.tensor.matmul(
                                ps[:],
                                w_sb[0][:, :, kk, co * 128:(co + 1) * 128],
                                rhs,
                                start=(kk == 0),
                                stop=(kk == 8),
                                perf_mode=PM.DoubleRow,
                            )
                        psv = ps[:].rearrange("p (r c) -> p r c", c=W)
                        nc.scalar.activation(
                            abv[:, 1 + RPC * s:1 + RPC * s + RPC, co, 1:57], psv, AF.Sign,
                            bias=t1_sb[:, co:co + 1], scale=s1_sb[:, co:co + 1],
                        )

                # ---- conv2 -> bn2 + residual + clip -> bf16 chunk stores ----
                for co in range(2):
                    xinv = xin[:, co].rearrange("p (h w) -> p h w", w=W)
                    # the very last chunk is split into two 4-row PSUM chunks
                    # so the final epilogue overlaps the preceding matmuls
                    chunks = [(RPC * s, RPC) for s in range(NCH)]
                    if n == NPER - 1 and co == 1:
                        chunks[-1:] = [(48, 4), (52, 4)]
                    for ci, (cr0, cnr) in enumerate(chunks):
                        ps = psp.tile([128, CHU], dt.float32, tag="ps")
                        for kk in range(9):
                            r0 = cr0 + kk // 3
                            rhs = abv[:, r0:r0 + cnr, :, kk % 3:kk % 3 + W].rearrange(
                                "p r k c -> p k r c")
                            nc.tensor.matmul(
                                ps[:, 0:cnr * W],
                                w_sb[1][:, :, kk, co * 128:(co + 1) * 128],
                                rhs,
                                start=(kk == 0),
                                stop=(kk == 8),
                                perf_mode=PM.DoubleRow,
                            )
                        psv = ps[:, 0:cnr * W].rearrange("p (r c) -> p r c", c=W)
                        tm = tmpp.tile([128, CHU], dt.float32, tag="tmp")
                        tmv = tm[:, 0:cnr * W].rearrange("p (r c) -> p r c", c=W)
                        nc.scalar.activation(
                            tmv, psv, AF.Identity,
                            bias=t2_sb[:, co:co + 1], scale=s2_sb[:, co:co + 1],
                        )
                        oc = ostp.tile([128, CHU], dt.bfloat16, tag="ost")
                        ocv = oc[:, 0:cnr * W].rearrange("p (r c) -> p r c", c=W)
                        nc.vector.tensor_tensor(
                            ocv, tmv, xinv[:, cr0:cr0 + cnr, :], ALU.add
                        )
                        nc.vector.tensor_scalar(
                            oc[:, 0:cnr * W], oc[:, 0:cnr * W],
                            1.0, -1.0, ALU.min, ALU.max)
                        st_eng = nc.sync if n == NPER - 1 else nc.gpsimd
                        st_eng.dma_start(
                            out_d[n, co, :, cr0 * W:(cr0 + cnr) * W],
                            oc[:, 0:cnr * W],
                        )

    nc.compile()
    return nc


def _get_nc():
    if "nc" not in _CACHE:
        _CACHE["nc"] = _build()
    return _CACHE["nc"]


def _prep_weights(w):
    # [co, cin, kh, kw] -> [cin 128, cin_chunk 2, tap 9, co 256], binarized fp8e4
    a = np.sign(w.astype(np.float32))
    a = a.transpose(1, 2, 3, 0).reshape(2, 128, 9, C).transpose(1, 0, 2, 3)
    return np.ascontiguousarray(a.astype(ml_dtypes.float8_e4m3))


def _fold_bn(g, b, m, v):
    s = (g.astype(np.float32) / np.sqrt(v.astype(np.float32) + BN_EPS)).astype(np.float32)
    t = (b.astype(np.float32) - m.astype(np.float32) * s).astype(np.float32)
    return (
        np.ascontiguousarray(s.reshape(2, 128).T),
        np.ascontiguousarray(t.reshape(2, 128).T),
    )


def _prep_x(x):
    """[64,256,56,56] fp32 -> padded binarized [64,2,128,58,2,64] fp8e4.

    Copy 0 holds image cols at c=1..56, copy 1 at c=2..57 (even-byte AP
    starts for every conv1 tap)."""
    n = x.shape[0]
    xb = np.zeros((n, 2, 128, PROWS, 2, ROWW), dtype=ml_dtypes.float8_e4m3)
    s = np.sign(x).astype(ml_dtypes.float8_e4m3)
    # [n, 2, 128, 56, 56] -> [n, 128, 56, 2, 56]
    s = s.reshape(n, 2, 128, H, W).transpose(0, 2, 3, 1, 4)
    xb[:, 0, :, 1:57, :, 1:57] = s
    xb[:, 1, :, 1:57, :, 2:58] = s
    return xb.reshape(n, 2, 128, PLSZ)


def _make_in_maps(inputs):
    x = inputs["x"]
    w1b = _prep_weights(inputs["w1"])
    w2b = _prep_weights(inputs["w2"])
    s1, t1 = _fold_bn(inputs["g1"], inputs["b1"], inputs["m1"], inputs["v1"])
    s2, t2 = _fold_bn(inputs["g2"], inputs["b2"], inputs["m2"], inputs["v2"])
    x = np.ascontiguousarray(x.astype(np.float32, copy=False))
    xb = _prep_x(x)
    res = x.reshape(N_CORES * NPER, 2, 128, HW)
    in_maps = []
    for c in range(N_CORES):
        in_maps.append({
            "xb": xb[c * NPER:(c + 1) * NPER],
            "res": res[c * NPER:(c + 1) * NPER],
            "w1b": w1b, "w2b": w2b,
            "s1": s1, "t1": t1, "s2": s2, "t2": t2,
        })
    return in_maps


def kernel(x, w1, g1, b1, m1, v1, w2, g2, b2, m2, v2):
    nc = _get_nc()
    in_maps = _make_in_maps({
        "x": x, "w1": w1, "g1": g1, "b1": b1, "m1": m1, "v1": v1,
        "w2": w2, "g2": g2, "b2": b2, "m2": m2, "v2": v2,
    })
    res = run_bass_kernel_spmd(nc, in_maps, list(range(N_CORES)))
    out = np.concatenate([res.results[c]["out"] for c in range(N_CORES)], axis=0)
    return out.astype(np.float32).reshape(N_CORES * NPER, C, H, W)
